# revision 4
# baseline (speedup 1.0000x reference)
"""Trainium2 Bass kernel for gnn_message_passing nn_CNNTest_10299331576114.

Strategy (V=100000 vertices sharded over 8 cores, 12500 each):

Stage 1 (NEFF-1): per core, gather g = vp[nb1] (12500x32 scalar indirect-DMA
gather), conv-k3 over the neighbor axis as a small banded matmul (host-packed
[33,32] matrix incl. bias row), relu, sum over neighbors -> h_raw shard
(mean's 1/32 is folded into downstream matrices).

Host: concat shards, build zero-padded gather table hp[100002].

Stage 2 (NEFF-2): per core, for each (v,j) gather the TRIPLE
(h[u-1], h[u], h[u+1]) = hp[u..u+2] where u = nb2[v,j] (12 B per index instead
of a 128 B f1 row - all the f1/conv math is linear pre-relu, so it is folded
into one host-packed [97,1024] matrix). Then:
  C = [T|1] @ Mbig   (PE),  relu (ACT),  h2 = sum_j (DVE reduce)
  h2^T written into a [33, 12502] vertex strip (PE transpose), ones row for
  biases, edge columns masked by a per-core input mask.
  f2^T = sum_k w2k^T @ strip_shift_k (PE, PSUM accum), logits = [f2|1]^T@wfcb,
  softmax via ACT exp with accumulated row-sum + DVE reciprocal/scale.
"""

import time

import numpy as np

import concourse.bacc as bacc
import concourse.mybir as mybir
import concourse.tile as tile
from concourse import bass
from concourse.bass import IndirectOffsetOnAxis
from concourse.bass_utils import run_bass_kernel_spmd
from concourse.masks import make_identity

F32 = mybir.dt.float32
I32 = mybir.dt.int32
AX = mybir.AxisListType
ALU = mybir.AluOpType
ACTF = mybir.ActivationFunctionType

V = 100000
N = 32
NCORES = 8
VC = V // NCORES          # 12500
P = 125                   # vertices per tile (partition dim)
T1 = VC // P              # 100 tiles per core
EXT = VC + 2              # stage-2 extended range (one halo vertex each side)
TA = T1 + 1               # 101 stage-2 gather tiles (last overlaps)

_CACHE = {}
TIMES = {}


def _build_stage1():
    nc = bacc.Bacc("TRN2", target_bir_lowering=False, debug=False,
                   num_devices=NCORES)
    vp = nc.dram_tensor("vp", [V], F32, kind="ExternalInput")
    nb1 = nc.dram_tensor("nb1", [VC, N], I32, kind="ExternalInput")
    a1 = nc.dram_tensor("a1", [N + 1, N], F32, kind="ExternalInput")
    hsh = nc.dram_tensor("hsh", [VC], F32, kind="ExternalOutput")

    with tile.TileContext(nc) as tc:
        with (
            tc.tile_pool(name="const", bufs=1) as cp,
            tc.tile_pool(name="io", bufs=4) as iop,
            tc.tile_pool(name="work", bufs=4) as wp,
            tc.tile_pool(name="hc", bufs=1) as hcp,
            tc.tile_pool(name="ps", bufs=2, space="PSUM") as psp,
            tc.tile_pool(name="psb", bufs=1, space="PSUM") as psb,
        ):
            ident = cp.tile([128, 128], F32)
            make_identity(nc, ident[:])
            a1t = cp.tile([N + 1, N], F32)
            nc.sync.dma_start(a1t[:], a1[:])
            hcol = hcp.tile([P, T1], F32)

            for t in range(T1):
                it = iop.tile([P, N], I32, tag="idx")
                nc.sync.dma_start(it[:], nb1[bass.ts(t, P), :])
                g = wp.tile([P, N], F32, tag="g")
                nc.gpsimd.indirect_dma_start(
                    out=g[:], out_offset=None, in_=vp[:, None],
                    in_offset=IndirectOffsetOnAxis(ap=it[:], axis=0))
                gtp = psp.tile([N, P], F32, tag="gt")
                nc.tensor.transpose(gtp[:], g[:], ident[:P, :P])
                gt = wp.tile([N + 1, P], F32, tag="gts")
                nc.vector.tensor_copy(gt[:N, :], gtp[:])
                nc.vector.memset(gt[N:N + 1, :], 1.0)
                c1p = psp.tile([P, N], F32, tag="c1")
                nc.tensor.matmul(c1p[:], lhsT=gt[:], rhs=a1t[:],
                                 start=True, stop=True)
                r = wp.tile([P, N], F32, tag="r")
                nc.scalar.activation(r[:], c1p[:], ACTF.Relu)
                nc.vector.reduce_sum(hcol[:, t:t + 1], r[:], axis=AX.X)

            htp = psb.tile([T1, P], F32)
            nc.tensor.transpose(htp[:], hcol[:], ident[:P, :P])
            hst = wp.tile([T1, P], F32, tag="hst")
            nc.vector.tensor_copy(hst[:], htp[:])
            nc.sync.dma_start(
                hsh[:].rearrange("(t p) -> t p", p=P), hst[:])
    nc.finalize()
    return nc


def _build_stage2():
    nc = bacc.Bacc("TRN2", target_bir_lowering=False, debug=False,
                   num_devices=NCORES)
    hp = nc.dram_tensor("hp", [V + 2], F32, kind="ExternalInput")
    nb2e = nc.dram_tensor("nb2e", [EXT, N], I32, kind="ExternalInput")
    mbig = nc.dram_tensor("mbig", [97, 1024], F32, kind="ExternalInput")
    w2k3 = nc.dram_tensor("w2k3", [3, 33, 64], F32, kind="ExternalInput")
    wfcb = nc.dram_tensor("wfcb", [65, 512], F32, kind="ExternalInput")
    mask2 = nc.dram_tensor("mask2", [32, 2], F32, kind="ExternalInput")
    out = nc.dram_tensor("out", [VC, 512], F32, kind="ExternalOutput")

    with tile.TileContext(nc) as tc:
        with (
            tc.tile_pool(name="const", bufs=1) as cp,
            tc.tile_pool(name="strip", bufs=1) as sp,
            tc.tile_pool(name="io", bufs=4) as iop,
            tc.tile_pool(name="work", bufs=4) as wp,
            tc.tile_pool(name="big", bufs=3) as bp,
            tc.tile_pool(name="psc", bufs=2, space="PSUM") as psc,
            tc.tile_pool(name="pst", bufs=2, space="PSUM") as pst,
            tc.tile_pool(name="psl", bufs=2, space="PSUM") as psl,
        ):
            ident = cp.tile([128, 128], F32)
            make_identity(nc, ident[:])
            mbigt = cp.tile([97, 1024], F32)
            nc.sync.dma_start(mbigt[:], mbig[:])
            w2kt = []
            for k in range(3):
                w2tile = cp.tile([33, 64], F32, tag=f"w2k{k}")
                nc.sync.dma_start(w2tile[:], w2k3[k])
                w2kt.append(w2tile)
            wfcbt = cp.tile([65, 512], F32)
            nc.sync.dma_start(wfcbt[:], wfcb[:])
            m2t = cp.tile([32, 2], F32)
            nc.sync.dma_start(m2t[:], mask2[:])

            strip = sp.tile([33, EXT], F32)
            nc.vector.memset(strip[32:33, :], 1.0)

            def phase_a(t):
                ot = min(P * t, EXT - P)
                it = iop.tile([P, N], I32, tag="idx")
                nc.sync.dma_start(it[:], nb2e[ot:ot + P, :])
                tt = wp.tile([P, 3 * N], F32, tag="tt")
                nc.gpsimd.indirect_dma_start(
                    out=tt[:], out_offset=None, in_=hp[:, None],
                    in_offset=IndirectOffsetOnAxis(ap=it[:], axis=0))
                ttp = pst.tile([96, P], F32, tag="tp")
                nc.tensor.transpose(ttp[:], tt[:], ident[:P, :P])
                tts = wp.tile([97, P], F32, tag="tts")
                nc.vector.tensor_copy(tts[:96, :], ttp[:])
                nc.vector.memset(tts[96:97, :], 1.0)
                cps = psc.tile([P, 1024], F32, tag="c")
                nc.tensor.matmul(cps[:, 0:512], lhsT=tts[:],
                                 rhs=mbigt[:, 0:512], start=True, stop=True)
                nc.tensor.matmul(cps[:, 512:1024], lhsT=tts[:],
                                 rhs=mbigt[:, 512:1024], start=True, stop=True)
                cr = bp.tile([P, 1024], F32, tag="cr")
                nc.scalar.activation(cr[:], cps[:], ACTF.Relu)
                h2 = wp.tile([P, N], F32, tag="h2")
                nc.vector.reduce_sum(
                    h2[:], cr[:].rearrange("p (c j) -> p c j", j=32),
                    axis=AX.X)
                h2p = pst.tile([N, P], F32, tag="tp")
                nc.tensor.transpose(h2p[:], h2[:], ident[:P, :P])
                nc.vector.tensor_copy(strip[0:32, ot:ot + P], h2p[:])
                if t == 0:
                    nc.vector.tensor_tensor(
                        out=strip[0:32, 0:1], in0=strip[0:32, 0:1],
                        in1=m2t[:, 0:1], op=ALU.mult)
                if t == TA - 1:
                    nc.vector.tensor_tensor(
                        out=strip[0:32, EXT - 1:EXT],
                        in0=strip[0:32, EXT - 1:EXT],
                        in1=m2t[:, 1:2], op=ALU.mult)

            def phase_b(t):
                f2p = pst.tile([64, P], F32, tag="tp")
                for k in range(3):
                    nc.tensor.matmul(
                        f2p[:], lhsT=w2kt[k][:],
                        rhs=strip[:, P * t + k:P * t + k + P],
                        start=(k == 0), stop=(k == 2))
                f2s = wp.tile([65, P], F32, tag="f2s")
                nc.vector.tensor_copy(f2s[:64, :], f2p[:])
                nc.vector.memset(f2s[64:65, :], 1.0)
                lgp = psl.tile([P, 512], F32, tag="lg")
                nc.tensor.matmul(lgp[:], lhsT=f2s[:], rhs=wfcbt[:],
                                 start=True, stop=True)
                e = bp.tile([P, 512], F32, tag="e")
                ssum = wp.tile([P, 1], F32, tag="ss")
                nc.scalar.activation(e[:], lgp[:], ACTF.Exp,
                                     accum_out=ssum[:])
                rinv = wp.tile([P, 1], F32, tag="ri")
                nc.vector.reciprocal(rinv[:], ssum[:])
                o = bp.tile([P, 512], F32, tag="o")
                nc.vector.tensor_scalar(out=o[:], in0=e[:], scalar1=rinv[:],
                                        scalar2=None, op0=ALU.mult)
                nc.sync.dma_start(out[bass.ts(t, P), :], o[:])

            phase_a(0)
            for t in range(1, TA):
                phase_a(t)
                phase_b(t - 1)
            phase_b(T1 - 1)
    nc.finalize()
    return nc


def _host_mats(wv1, bv1, w1, b1, wv2, bv2, w2, b2, wfc, bfc):
    w1m = w1[:, 0, :].astype(np.float32)                    # [32, 3]
    a1 = np.zeros((N + 1, N), np.float32)                   # stage-1 conv
    for j in range(N):
        for dj in range(3):
            jp = j - 1 + dj
            if 0 <= jp < N:
                a1[jp, j] = wv1[dj]
    a1[N, :] = bv1[0]

    mbig = np.zeros((97, 1024), np.float32)
    cidx = np.arange(32) * 32
    for j in range(32):
        for dj in range(3):
            jp = j - 1 + dj
            if 0 <= jp < 32:
                for dk in range(3):
                    mbig[jp * 3 + dk, cidx + j] = wv2[dj] * w1m[:, dk] / 32.0
    for j in range(32):
        s = sum(wv2[dj] for dj in range(3) if 0 <= j - 1 + dj < 32)
        mbig[96, cidx + j] = bv2[0] + b1 * s

    w2k3 = np.zeros((3, 33, 64), np.float32)
    for k in range(3):
        w2k3[k, :32, :] = w2[:, :, k].T / 32.0
    w2k3[0, 32, :] = b2                                     # bias only on k=0

    wfcb = np.zeros((65, 512), np.float32)
    wfcb[:64] = wfc.T
    wfcb[64] = bfc
    return a1, mbig, w2k3, wfcb


def kernel(vp, nb1, nb2, wv1, bv1, w1, b1, wv2, bv2, w2, b2, wfc, bfc):
    vp = np.ascontiguousarray(np.asarray(vp, dtype=np.float32))
    nb1 = np.ascontiguousarray(np.asarray(nb1).astype(np.int32))
    nb2 = np.ascontiguousarray(np.asarray(nb2).astype(np.int32))
    wv1 = np.asarray(wv1, np.float32); bv1 = np.asarray(bv1, np.float32)
    w1 = np.asarray(w1, np.float32); b1 = np.asarray(b1, np.float32)
    wv2 = np.asarray(wv2, np.float32); bv2 = np.asarray(bv2, np.float32)
    w2 = np.asarray(w2, np.float32); b2 = np.asarray(b2, np.float32)
    wfc = np.asarray(wfc, np.float32); bfc = np.asarray(bfc, np.float32)

    a1, mbig, w2k3, wfcb = _host_mats(wv1, bv1, w1, b1, wv2, bv2, w2, b2,
                                      wfc, bfc)

    if "s1" not in _CACHE:
        _CACHE["s1"] = _build_stage1()
    if "s2" not in _CACHE:
        _CACHE["s2"] = _build_stage2()

    core_ids = list(range(NCORES))

    # ---- stage 1 ----
    in1 = [{"vp": vp, "nb1": nb1[VC * c:VC * (c + 1)], "a1": a1}
           for c in range(NCORES)]
    t0 = time.time()
    res1 = run_bass_kernel_spmd(_CACHE["s1"], in1, core_ids=core_ids)
    TIMES["stage1_wall"] = time.time() - t0
    hp = np.zeros(V + 2, np.float32)
    for c in range(NCORES):
        hp[1 + VC * c:1 + VC * (c + 1)] = res1.results[c]["hsh"]

    # ---- stage 2 ----
    in2 = []
    for c in range(NCORES):
        vstart = VC * c
        nb2e = np.zeros((EXT, N), np.int32)
        lo = max(vstart - 1, 0)
        hi = min(vstart + VC + 1, V)
        nb2e[lo - (vstart - 1):hi - (vstart - 1)] = nb2[lo:hi]
        mask2 = np.ones((32, 2), np.float32)
        if c == 0:
            mask2[:, 0] = 0.0
        if c == NCORES - 1:
            mask2[:, 1] = 0.0
        in2.append({"hp": hp, "nb2e": nb2e, "mbig": mbig, "w2k3": w2k3,
                    "wfcb": wfcb, "mask2": mask2})
    t0 = time.time()
    res2 = run_bass_kernel_spmd(_CACHE["s2"], in2, core_ids=core_ids)
    TIMES["stage2_wall"] = time.time() - t0
    return np.concatenate([res2.results[c]["out"] for c in range(NCORES)],
                          axis=0)
